# revision 31
# baseline (speedup 1.0000x reference)
"""AlmostFairKCRPSLoss (alpha=1) on 8 TRN2 NeuronCores.

Math (per pixel, m=16 ensemble members x_i, target y):
  skill  = (1/16) sum_i |x_i - y|
  spread = (1/480) sum_{i,j} |x_i - x_j| = (1/240) sum_{i<j} |x_i - x_j|
  out    = mean_px (skill - spread)

Using |a-b| = 2*max(a,b) - a - b, the sum_i x_i terms cancel between skill
and spread, leaving per pixel:
  skill - spread = (1/8)*sum_i max(x_i,y) - (1/120)*sum_{i<j} max(x_i,x_j) - y

Only SUMS OF PAIRWISE MAXES are needed. Engine split per core:
  - VectorE: all maxes via bf16 tensor_max (2x DVE mode). Spread = offset
    sweeps d=1..15 over the member block (120 pairs); skill = 9 small TTs of
    1-2 members vs a stride-0-broadcast target, used as filler while DMAs
    trickle in.
  - TensorE: reduces the spread max tiles with ones-vector matmuls
    accumulated into one PSUM slice.
  - ScalarE: f32->bf16 casts, skill-max reduction via activation accum_out,
    the exact f32 target sum, and the final PSUM->SBUF copy.
Host applies the 1/8 and 1/120 weights and the global mean.

Sharding: pure data parallel over the flat pixel volume: 663552 px / 8 cores
= 82944 px/core = 128 partitions x 648 free.
"""

import os

import numpy as np

# The axon trace path needs an NTFF hook that is absent in this container;
# make sure a stray BASS_TRACE env var cannot route us onto it.
os.environ.setdefault("BASS_NEVER_TRACE", "1")

import concourse.bass as bass
import concourse.bacc as bacc
import concourse.mybir as mybir
from concourse import tile
from concourse.bass_utils import run_bass_kernel_spmd

P = 128            # SBUF partitions
F = 648            # pixels per partition per core
M = 16             # ensemble size
NCORES = 8
NPIX = P * F       # 82944 pixels per core
NPIX_TOTAL = NPIX * NCORES  # 663552
MMCHUNK = 512      # matmul moving free-dim chunk (one PSUM bank)
NSK = 9            # skill TT groups: (0),(15,1),(14,2),...,(9,7),(8)
NACC = NSK + 2     # + target sum col, + ACT-reduced spread tail col

_f32 = mybir.dt.float32
_bf16 = mybir.dt.bfloat16


def _member_order():
    order = []
    lo, hi = 0, M - 1
    while lo <= hi:
        order.append(lo)
        if hi != lo:
            order.append(hi)
        lo += 1
        hi -= 1
    return order


def _sweep_pieces():
    """(d, p0_block, p1_block) emission list: d=15..9 during arrivals, then
    8..1; sweeps with >8 blocks split so PSUM reduction chases closely and
    the final piece is tiny."""
    pieces = []
    for j in range(1, 8):
        pieces.append((M - j, 0, j))
    for d in range(8, 0, -1):
        nblk = M - d
        if nblk <= 8:
            pieces.append((d, 0, nblk))
        elif d > 1:
            pieces.append((d, 0, 8))
            pieces.append((d, 8, nblk))
        else:
            pieces.append((1, 0, 8))
            pieces.append((1, 8, 14))
            pieces.append((1, 14, 15))
    return pieces


def build_graph(loop_k=None):
    nc = bacc.Bacc(
        "TRN2", target_bir_lowering=False, debug=False, num_devices=NCORES
    )
    pred_d = nc.dram_tensor("pred", [M, NPIX], _f32, kind="ExternalInput")
    tgt_d = nc.dram_tensor("target", [1, NPIX], _f32, kind="ExternalInput")
    outp_d = nc.dram_tensor("outp", [1, MMCHUNK], _f32, kind="ExternalOutput")
    outa_d = nc.dram_tensor("outa", [P, NACC], _f32, kind="ExternalOutput")

    pred_ap = pred_d.ap().rearrange("m (p f) -> m p f", p=P)
    tgt_ap = tgt_d.ap().rearrange("o (p f) -> o p f", p=P)
    order = _member_order()
    pieces = _sweep_pieces()

    sp_chunks = []   # (d, p0, c0, c1) 512-col matmul chunks, emission order
    for (d, b0, b1) in pieces:
        if (d, b0, b1) == (1, 14, 15):
            continue   # reduced on ScalarE instead
        c = b0 * F
        while c < b1 * F:
            e = min(c + MMCHUNK, b1 * F)
            sp_chunks.append((d, b0, c, e))
            c = e

    with tile.TileContext(nc) as tc:
        with (
            tc.tile_pool(name="main", bufs=1) as pool,
            tc.tile_pool(name="mx", bufs=3) as mxpool,
            tc.tile_pool(name="mxs", bufs=9) as mxspool,
            tc.tile_pool(name="ps", bufs=1, space="PSUM") as pspool,
        ):
            stage = pool.tile([P, (M + 1) * F], _f32)   # slot 16 = target
            mb = pool.tile([P, (M + 1) * F], _bf16)
            ones = pool.tile([P, 1], _bf16)
            acc = pool.tile([P, NACC], _f32)
            outb = pool.tile([1, MMCHUNK], _f32)
            psum_sp = pspool.tile([1, MMCHUNK], _f32)

            nc.vector.memset(ones[:, :], 1.0)

            import contextlib
            loop_ctx = (
                tc.For_i(0, loop_k, 1) if loop_k else contextlib.nullcontext()
            )

            def cast(m):
                nc.scalar.copy(
                    out=mb[:, bass.ts(m, F)], in_=stage[:, bass.ts(m, F)]
                )

            skill_accums = []

            def emit_skill(g, members):
                nb = len(members)
                src = stage if g < 2 else mb   # first groups: f32, no cast dep
                if g < 2:
                    mx = mxspool.tile([P, 2 * F], _f32, tag="mxsf")
                else:
                    mx = mxspool.tile([P, 2 * F], _bf16, tag="mxs")
                if nb == 1:
                    in0 = src[:, bass.ts(members[0], F)].unsqueeze(1)
                else:
                    lo, hi = min(members), max(members)
                    in0 = (
                        src[:, lo * F : (hi + 1) * F]
                        .rearrange("p (m f) -> p m f", f=F)[:, :: (hi - lo), :]
                    )
                in1 = src[:, bass.ts(M, F)].unsqueeze(1).broadcast_to((P, nb, F))
                out3 = mx[:, 0 : nb * F].rearrange("p (m f) -> p m f", f=F)
                nc.vector.tensor_max(out3, in0, in1)
                # skill reduction deferred to ScalarE after all casts
                skill_accums.append((g, nb, mx))

            def emit_sweep_piece(piece):
                d, b0, b1 = piece
                p0, p1 = b0 * F, b1 * F
                mx = mxpool.tile([P, 8 * F], _bf16, tag="mx")
                nc.vector.tensor_max(
                    mx[:, 0 : p1 - p0], mb[:, p0:p1], mb[:, d * F + p0 : d * F + p1]
                )
                if (d, b0, b1) == (1, 14, 15):
                    nc.scalar.activation(
                        out=mx[:, 0 : p1 - p0],
                        in_=mx[:, 0 : p1 - p0],
                        func=mybir.ActivationFunctionType.Identity,
                        accum_out=acc[:, NSK + 1 : NSK + 2],
                    )
                    return
                for (dd, bb, c0, c1) in sp_chunks:
                    if dd != d or bb != b0:
                        continue
                    nc.tensor.matmul(
                        psum_sp[:, 0 : c1 - c0],
                        ones[:, :],
                        mx[:, c0 - p0 : c1 - p0],
                        start=(dd, bb, c0, c1) == sp_chunks[0],
                        stop=(dd, bb, c0, c1) == sp_chunks[-1],
                    )

            # ---- target: DMA, cast, exact f32 sum on ScalarE ----
            loop_ctx.__enter__()
            nc.sync.dma_start(out=stage[:, bass.ts(M, F)], in_=tgt_ap[0])
            cast(M)
            nc.scalar.activation(
                out=stage[:, bass.ts(M, F)],
                in_=stage[:, bass.ts(M, F)],
                func=mybir.ActivationFunctionType.Identity,
                accum_out=acc[:, NSK : NSK + 1],
            )

            # ---- members: DMA + cast + fillers; sweeps when operands landed
            sweep_iter = iter(pieces)
            emitted = 0
            skill_groups = [[0], [15, 1], [14, 2], [13, 3], [12, 4],
                            [11, 5], [10, 6], [9, 7], [8]]
            gnext = 0
            arrived = set()
            for k, m in enumerate(order):
                nc.sync.dma_start(out=stage[:, bass.ts(m, F)], in_=pred_ap[m])
                cast(m)
                arrived.add(m)
                while gnext < NSK and all(
                    x in arrived for x in skill_groups[gnext]
                ):
                    emit_skill(gnext, skill_groups[gnext])
                    gnext += 1
                if k % 2 == 1 and emitted < 7:
                    emit_sweep_piece(next(sweep_iter))
                    emitted += 1
            # exact f32 target sum (dummy out so stage stays read-only)
            tsdump = mxspool.tile([P, 2 * F], _f32, tag="mxsf")
            nc.scalar.activation(
                out=tsdump[:, 0:F],
                in_=stage[:, bass.ts(M, F)],
                func=mybir.ActivationFunctionType.Identity,
                accum_out=acc[:, NSK : NSK + 1],
            )
            for g, nb, mx in skill_accums:
                nc.scalar.activation(
                    out=mx[:, 0 : nb * F],
                    in_=mx[:, 0 : nb * F],
                    func=mybir.ActivationFunctionType.Identity,
                    accum_out=acc[:, g : g + 1],
                )
            for piece in sweep_iter:
                emit_sweep_piece(piece)

            nc.scalar.copy(out=outb[:, :], in_=psum_sp[:, :])
            nc.sync.dma_start(out=outp_d.ap(), in_=outb[:, :])
            nc.sync.dma_start(out=outa_d.ap(), in_=acc[:, :])
            loop_ctx.__exit__(None, None, None)

    nc.compile()
    return nc


_GRAPH = None


def _get_graph():
    global _GRAPH
    if _GRAPH is None:
        _GRAPH = build_graph()
    return _GRAPH


def run(target, pred, **spmd_kwargs):
    """Returns (scalar_result, BassKernelResults)."""
    target = np.ascontiguousarray(target, dtype=np.float32).reshape(1, NPIX_TOTAL)
    pred = np.ascontiguousarray(pred, dtype=np.float32).reshape(M, NPIX_TOTAL)
    in_maps = []
    for r in range(NCORES):
        sl = slice(r * NPIX, (r + 1) * NPIX)
        in_maps.append(
            {
                "pred": np.ascontiguousarray(pred[:, sl]),
                "target": np.ascontiguousarray(target[:, sl]),
            }
        )
    nc = _get_graph()
    try:
        res = run_bass_kernel_spmd(nc, in_maps, list(range(NCORES)), **spmd_kwargs)
    except Exception:
        # transient device errors have been observed on this pool; retry once
        res = run_bass_kernel_spmd(nc, in_maps, list(range(NCORES)), **spmd_kwargs)
    total = 0.0
    for r in range(NCORES):
        oa = res.results[r]["outa"].astype(np.float64)
        sp = res.results[r]["outp"].astype(np.float64).sum() + oa[:, NSK + 1].sum()
        sk = oa[:, 0:NSK].sum()
        tg = oa[:, NSK].sum()
        total += sk / 8.0 - sp / 120.0 - tg
    return np.array(total / NPIX_TOTAL, dtype=np.float32), res


def kernel(target, pred):
    value, _ = run(target, pred)
    return value
